# revision 12
# baseline (speedup 1.0000x reference)
"""Trainium2 Bass kernel: contrastive loss with negative mining (v4).

Math:
    centers  = mean over contiguous chunks of 8 rows               [n/8, d]
    x_pos    = x + 0.5*(center - x)        => |x - x_pos| = 0.5*|x - center|
    sim      = x @ x.T                                             [n, n]
    neg_idx  = argmax_j sim[i, j] excluding j in i's group-of-4
    d_ap     = mean_d |x - x_pos|,  d_an = mean_d |x - x_neg|
    loss     = sum( (1/8) * d_ap / (d_an + 1e-7) )

Distribution: data-parallel over rows, 8 NeuronCores, 1024 rows each,
no collectives; per-row losses summed on host.

v4 strategy (v3 + wide PSUM groups, power-aware engine balance):
  - Per-core xm drops the core's own 1024 columns (14 strips); the
    diagonal block comes from the resident xs slice and is processed
    first so the PE starts ~2us after launch while xm streams in.  The
    group-of-4 exclusion window always falls in the diagonal block
    where the python loop knows the i-tile, so masks are static bf16
    tiles.
  - Sim strips are accumulated into wide PSUM tiles (up to 4 strips =
    4 banks), evacuated by one wide ScalarE copy to bf16 and mined by
    one wide DVE max-reduce -- minimizing per-strip instruction count
    and data traffic, which keeps the chip off the power throttle that
    otherwise slows the PE clock.
  - Strip buffers are DMA'd to a DRAM scratch; combine finds the
    winning slot via MAX8+max_index over 16 per-strip maxima, gathers
    each row's winning strip back, and recovers the column with one
    MAX8+max_index on it.  u32 arithmetic maps (slot, col) to the
    global index.
  - d_ap = |M2 @ x| uses bf16 matmuls + a single DVE abs-add-reduce
    straight out of PSUM (no ScalarE involvement).
"""

import math

import ml_dtypes
import numpy as np

import concourse.bass as bass
import concourse.mybir as mybir
import concourse.tile as tile
from concourse import bacc
from concourse.bass import IndirectOffsetOnAxis
from concourse.bass_utils import run_bass_kernel_spmd

BF16 = mybir.dt.bfloat16
F32 = mybir.dt.float32
U32 = mybir.dt.uint32
ALU = mybir.AluOpType
ACTF = mybir.ActivationFunctionType
AXX = mybir.AxisListType.X

P = 128         # partitions / row-tile height
JS = 512        # similarity column-strip width
CHUNK = 8       # rows averaged per center
GROUP = 4       # negative-mining exclusion window
WEIGHT = 1.0 / 8
EPS = 1e-7
MASKV = -float(2 ** 26)           # exclusion-window additive mask

# strip-slot groups per i-tile: [diag0, diag1], then regular strips in
# fours; slot s (0..15) maps to global strip 2c+s for s<2, else packed
# strip j=s-2 whose global id is j + 2*(j >= 2c).
SGROUPS = [(0, 2), (2, 6), (6, 10), (10, 14), (14, 16)]


class Cfg:
    def __init__(self, n=8192, d=2048, cores=8, fp8=True):
        self.n, self.d, self.cores, self.fp8 = n, d, cores, fp8
        self.r = n // cores            # rows per core
        self.it = self.r // P          # i-tiles per core
        self.nj = n // JS              # global column strips
        self.njr = self.nj - 2         # regular (non-diagonal) strips
        self.kb = d // P               # contraction blocks
        self.gi = min(4, self.it)      # i-tiles per pass
        assert n % (cores * P) == 0 and d % P == 0 and n % JS == 0
        assert self.it % self.gi == 0
        assert self.r == 2 * JS        # diag block is exactly 2 strips
        assert self.d == 4 * JS        # d_ap fits one wide PSUM tile


def _body(tc: tile.TileContext, cfg: Cfg, io: dict):
    nc = tc.nc
    ctxpools = {}

    def pool(name, bufs, space="SBUF"):
        if name not in ctxpools:
            ctxpools[name] = tc.alloc_tile_pool(name=name, bufs=bufs, space=space)
        return ctxpools[name]

    sim_dt = mybir.dt.float8e4 if cfg.fp8 else BF16
    NS = cfg.nj          # slots per i-tile
    G = cfg.gi

    # resident stationary xT slice: [128, KB*R], k-block major.
    # Chunked DMAs so the diag matmuls start before the full load lands.
    xs_sb = pool("xs", 1).tile([P, cfg.kb * cfg.r], sim_dt, name="xs_sb")
    for k in range(0, cfg.kb, 2):
        ke = min(k + 2, cfg.kb)
        nc.sync.dma_start(
            out=xs_sb[:, k * cfg.r:ke * cfg.r].rearrange(
                "p (a r) -> p a r", a=ke - k),
            in_=io["xs"][k * P:ke * P, :].rearrange("(a p) r -> p a r", p=P),
        )

    consts = pool("consts", 1)
    maskdg_sb = consts.tile_from(io["maskdg"])   # [128, 4*512] bf16
    prow_sb = consts.tile_from(io["prow"])       # [128, 8] u32
    ctab2_sb = consts.tile_from(io["ctab2"])     # [128, 8] u32 (value 2c)
    m2b_sb = consts.tile_from(io["m2b"])         # [128,128] bf16

    # resident bf16 x rows (d_an minuend / d_ap input); only needed by the
    # pass tails -- its DMA is emitted mid-pass so xm strips go first.
    xrb_sb = pool("xrb", 1).tile([P, cfg.it * cfg.d], BF16, name="xrb_sb")
    xrb_loaded = [False]

    def load_xrb():
        if not xrb_loaded[0]:
            xrb_loaded[0] = True
            nc.sync.dma_start(
                out=xrb_sb[:].rearrange("p (a d) -> p a d", a=cfg.it),
                in_=io["xrb"][:, :].rearrange("(a p) d -> p a d", p=P),
            )

    psum = pool("ps", 2, space="PSUM")
    small = pool("small", 1)
    san = small.tile([P, cfg.it], F32, name="san")             # sum|x-xneg|
    sap = small.tile([P, cfg.it * 4], F32, name="sap")         # sum|y| per chunk
    sapS = small.tile([P, cfg.gi], F32, name="sapS")           # sum|y| (scalar path)
    idxall = small.tile([P, cfg.it], U32, name="idxall")       # neg indices
    npass = cfg.it // G
    cands = [small.tile([P, G * NS], BF16, name=f"cd{g}", tag=f"cd{g}")
             for g in range(npass)]

    xmp = pool("xm", 6)
    sskp = pool("ssk", 4)
    comb = pool("comb", 1)
    segp = pool("seg", 2)
    xneg_p = pool("xneg", 2)
    diff_p = pool("diff", 2)
    dabs_p = pool("dabs", 2)

    xs3 = xs_sb[:].rearrange("p (a r) -> p a r", a=cfg.kb)
    simr3 = io["simr"][:, :].rearrange("(p a) c -> p a c", p=P)  # [128,128,512]

    def mm_quarter(ps_s, q, it, rhs_fp8, rhs_bf16):
        """One strip's contraction into quarter q of a wide PSUM tile."""
        if cfg.fp8:
            for k in range(0, cfg.kb, 2):
                nc.tensor.matmul(
                    out=ps_s[:, q * JS:(q + 1) * JS],
                    lhsT=xs3[:, k:k + 2, it * P:(it + 1) * P],
                    rhs=rhs_fp8(k),
                    start=(k == 0), stop=(k == cfg.kb - 2),
                    perf_mode=mybir.MatmulPerfMode.DoubleRow,
                )
        else:
            for k in range(cfg.kb):
                nc.tensor.matmul(
                    out=ps_s[:, q * JS:(q + 1) * JS],
                    lhsT=xs_sb[:, k * cfg.r + it * P:k * cfg.r + (it + 1) * P],
                    rhs=rhs_bf16(k),
                    start=(k == 0), stop=(k == cfg.kb - 1),
                )

    for a in range(0, cfg.it, G):
        g = a // G
        ssks = {}
        xms = {}
        for it in range(a, a + G):
            ssks[it] = sskp.tile([P, NS * JS], BF16, name="ssk")
        for (s0, s1) in SGROUPS:
            w = s1 - s0
            # DMA this group's regular strips (diag comes from xs)
            for s in range(max(s0, 2), s1):
                j = s - 2
                xm_sb = xmp.tile([P, cfg.kb * JS], sim_dt, name="xm_sb")
                xms[s] = xm_sb
                nc.sync.dma_start(
                    out=xm_sb[:].rearrange("p (a b) -> p a b", a=cfg.kb),
                    in_=io["xm"][:, j * JS:(j + 1) * JS].rearrange(
                        "(a p) b -> p a b", p=P),
                )
            if s0 == 2:
                load_xrb()
            for it in range(a, a + G):
                ps_s = psum.tile([P, 4 * JS], F32, name="ps_s", tag="ps")
                for q in range(w):
                    s = s0 + q
                    if s < 2:
                        mm_quarter(
                            ps_s, q, it,
                            lambda k, s=s: xs3[:, k:k + 2, s * JS:(s + 1) * JS],
                            lambda k, s=s: xs_sb[:, k * cfg.r + s * JS:
                                                 k * cfg.r + (s + 1) * JS],
                        )
                    else:
                        xm3 = xms[s][:].rearrange("p (a b) -> p a b", a=cfg.kb)
                        mm_quarter(
                            ps_s, q, it,
                            lambda k, xm3=xm3: xm3[:, k:k + 2, :],
                            lambda k, s=s: xms[s][:, k * JS:(k + 1) * JS],
                        )
                dst = ssks[it][:, s0 * JS:s1 * JS]
                nc.scalar.copy(out=dst, in_=ps_s[:, 0:w * JS])
                if s0 == 0:
                    # exclusion mask on the diag strip holding it's window
                    ds = it // 4
                    nc.vector.tensor_tensor(
                        out=ssks[it][:, ds * JS:(ds + 1) * JS],
                        in0=ssks[it][:, ds * JS:(ds + 1) * JS],
                        in1=maskdg_sb[:, (it % 4) * JS:(it % 4 + 1) * JS],
                        op=ALU.add)
                nc.vector.tensor_reduce(
                    out=cands[g][:, (it - a) * NS + s0:(it - a) * NS + s1],
                    in_=dst.rearrange("p (s c) -> p s c", s=w),
                    axis=AXX, op=ALU.max)
            if s1 == 10:
                for it in range(a, a + G):
                    nc.sync.dma_start(
                        out=simr3[:, it * NS:it * NS + 8, :],
                        in_=ssks[it][:, 0:8 * JS].rearrange(
                            "p (s c) -> p s c", s=8),
                    )
        for it in range(a, a + G):
            nc.sync.dma_start(
                out=simr3[:, it * NS + 8:(it + 1) * NS, :],
                in_=ssks[it][:, 8 * JS:].rearrange(
                    "p (s c) -> p s c", s=NS - 8),
            )

        # ---- combine: find winning slot, recover column via gather ----
        slotv = comb.tile([P, G], U32, name="slotv", tag=f"slotv{g}")
        for it in range(a, a + G):
            t8 = comb.tile([P, 8], BF16, name="t8")
            nc.vector.max(
                out=t8[:], in_=cands[g][:, (it - a) * NS:(it - a + 1) * NS])
            i8 = comb.tile([P, 8], U32, name="i8")
            nc.vector.max_index(
                out=i8[:], in_max=t8[:],
                in_values=cands[g][:, (it - a) * NS:(it - a + 1) * NS])
            nc.vector.tensor_copy(
                out=slotv[:, it - a:it - a + 1], in_=i8[:, 0:1])
        rowidv = comb.tile([P, G], U32, name="rowidv", tag=f"row{g}")
        nc.vector.tensor_tensor(
            out=rowidv[:], in0=prow_sb[:, a:a + G], in1=slotv[:], op=ALU.add)

        # global strip id: diag slots (s<2): 2c + s; else j=s-2,
        # g = j + 2*(j >= 2c)
        sl2 = comb.tile([P, G], U32, name="sl2")
        nc.vector.tensor_scalar(
            out=sl2[:], in0=slotv[:], scalar1=2, scalar2=None,
            op0=ALU.subtract)
        ge2 = comb.tile([P, G], U32, name="ge2")
        nc.vector.tensor_tensor(
            out=ge2[:], in0=sl2[:], in1=ctab2_sb[:, a:a + G], op=ALU.is_ge)
        ge2s = comb.tile([P, G], U32, name="ge2s")
        nc.vector.tensor_scalar(
            out=ge2s[:], in0=ge2[:], scalar1=1, scalar2=None,
            op0=ALU.logical_shift_left)
        gplus = comb.tile([P, G], U32, name="gplus")
        nc.vector.tensor_tensor(
            out=gplus[:], in0=sl2[:], in1=ge2s[:], op=ALU.add)
        dgv = comb.tile([P, G], U32, name="dgv")
        nc.vector.tensor_tensor(
            out=dgv[:], in0=slotv[:], in1=ctab2_sb[:, a:a + G], op=ALU.add)
        isd = comb.tile([P, G], U32, name="isd")
        nc.vector.tensor_scalar(
            out=isd[:], in0=slotv[:], scalar1=2, scalar2=None, op0=ALU.is_lt)
        gsel = comb.tile([P, G], U32, name="gsel", tag=f"gsel{g}")
        nc.vector.select(out=gsel[:], mask=isd[:], on_true=dgv[:],
                         on_false=gplus[:])
        gsh = comb.tile([P, G], U32, name="gsh", tag=f"gsh{g}")
        nc.vector.tensor_scalar(
            out=gsh[:], in0=gsel[:], scalar1=9, scalar2=None,
            op0=ALU.logical_shift_left)

        colv = comb.tile([P, G], U32, name="colv", tag=f"colv{g}")
        for it in range(a, a + G):
            seg = segp.tile([P, JS], BF16, name="seg")
            nc.gpsimd.indirect_dma_start(
                out=seg[:], out_offset=None,
                in_=io["simr"][:, :],
                in_offset=IndirectOffsetOnAxis(
                    ap=rowidv[:, it - a:it - a + 1], axis=0),
                bounds_check=P * P - 1, oob_is_err=False,
            )
            s8 = comb.tile([P, 8], BF16, name="s8")
            nc.vector.max(out=s8[:], in_=seg[:])
            c8 = comb.tile([P, 8], U32, name="c8")
            nc.vector.max_index(out=c8[:], in_max=s8[:], in_values=seg[:])
            nc.vector.tensor_copy(
                out=colv[:, it - a:it - a + 1], in_=c8[:, 0:1])
        nc.vector.tensor_tensor(
            out=idxall[:, a:a + G], in0=gsh[:], in1=colv[:], op=ALU.bitwise_or)

        # ---- gather x_neg (bf16) + d_an for this pass's i-tiles ----
        for it in range(a, a + G):
            xneg = xneg_p.tile([P, cfg.d], BF16, name="xneg")
            nc.gpsimd.indirect_dma_start(
                out=xneg[:], out_offset=None,
                in_=io["xfb"][:, :],
                in_offset=IndirectOffsetOnAxis(ap=idxall[:, it:it + 1], axis=0),
                # an OOB index must not fault the device; skip it instead
                bounds_check=cfg.n - 1, oob_is_err=False,
            )
            diff = diff_p.tile([P, cfg.d], BF16, name="diff")
            nc.vector.tensor_tensor(
                out=diff[:], in0=xrb_sb[:, it * cfg.d:(it + 1) * cfg.d],
                in1=xneg[:], op=ALU.subtract,
            )
            dabs = dabs_p.tile([P, cfg.d], BF16, name="dabs")
            nc.scalar.activation(
                out=dabs[:], in_=diff[:], func=ACTF.Abs,
                accum_out=san[:, it:it + 1],
            )

        # ---- d_ap for this pass's i-tiles: y = M2 @ x_tile, sum_d |y|.
        # Pass A uses a DVE abs-add-reduce (hidden under pass B's matmuls);
        # pass B uses ScalarE Abs+accum (hidden under the DVE mining tail).
        for it in range(a, a + G):
            ps_y = psum.tile([P, 4 * JS], F32, name="ps_y", tag="ps")
            for c in range(4):
                nc.tensor.matmul(
                    out=ps_y[:, c * JS:(c + 1) * JS], lhsT=m2b_sb[:],
                    rhs=xrb_sb[:, it * cfg.d + c * JS:
                               it * cfg.d + (c + 1) * JS],
                    start=True, stop=True,
                )
            if a == 0:
                nc.vector.tensor_reduce(
                    out=sap[:, it * 4:(it + 1) * 4],
                    in_=ps_y[:].rearrange("p (a b) -> p a b", a=4),
                    axis=AXX, op=ALU.add, apply_absolute_value=True,
                )
            else:
                yab = dabs_p.tile([P, cfg.d], BF16, name="dabs")
                nc.scalar.activation(
                    out=yab[:], in_=ps_y[:], func=ACTF.Abs,
                    accum_out=sapS[:, it - G:it - G + 1],
                )

    # ---- Final: per-row loss ----
    fin = pool("fin", 1)
    sap8 = fin.tile([P, cfg.it], F32, name="sap8")
    sap4h = sap[:, 0:4 * G].rearrange("p (a b) -> p a b", a=G)
    nc.vector.tensor_reduce(out=sap8[:, 0:G], in_=sap4h, axis=AXX, op=ALU.add)
    nc.vector.tensor_copy(out=sap8[:, G:2 * G], in_=sapS[:])
    assert cfg.it == 2 * G
    t1 = fin.tile([P, cfg.it], F32, name="t1")
    nc.vector.tensor_scalar(
        out=t1[:], in0=san[:], scalar1=1.0 / cfg.d, scalar2=EPS,
        op0=ALU.mult, op1=ALU.add,
    )
    rec = fin.tile([P, cfg.it], F32, name="rec")
    nc.vector.reciprocal(out=rec[:], in_=t1[:])
    t2 = fin.tile([P, cfg.it], F32, name="t2")
    nc.vector.tensor_tensor(out=t2[:], in0=sap8[:], in1=rec[:], op=ALU.mult)
    lossv = fin.tile([P, cfg.it], F32, name="lossv")
    nc.vector.tensor_scalar(
        out=lossv[:], in0=t2[:], scalar1=0.5 * WEIGHT / cfg.d, scalar2=None,
        op0=ALU.mult,
    )
    nc.sync.dma_start(out=io["loss_part"][:, :], in_=lossv[:])
    nc.sync.dma_start(out=io["nidx"][:, :], in_=idxall[:])

    for p in reversed(list(ctxpools.values())):
        p.release()


def build(cfg: Cfg) -> bass.Bass:
    nc = bacc.Bacc("TRN2", target_bir_lowering=False, debug=False)
    sim_dt = mybir.dt.float8e4 if cfg.fp8 else BF16
    io = {
        "xm": nc.dram_tensor("xm", [cfg.d, cfg.njr * JS], sim_dt,
                             kind="ExternalInput").ap(),
        "xs": nc.dram_tensor("xs", [cfg.d, cfg.r], sim_dt,
                             kind="ExternalInput").ap(),
        "xrb": nc.dram_tensor("xrb", [cfg.r, cfg.d], BF16,
                              kind="ExternalInput").ap(),
        "xfb": nc.dram_tensor("xfb", [cfg.n, cfg.d], BF16,
                              kind="ExternalInput").ap(),
        "m2b": nc.dram_tensor("m2b", [P, P], BF16, kind="ExternalInput").ap(),
        "maskdg": nc.dram_tensor("maskdg", [P, 4 * JS], BF16,
                                 kind="ExternalInput").ap(),
        "prow": nc.dram_tensor("prow", [P, 8], U32,
                               kind="ExternalInput").ap(),
        "ctab2": nc.dram_tensor("ctab2", [P, 8], U32,
                                kind="ExternalInput").ap(),
        "simr": nc.dram_tensor("simr", [P * P, JS], BF16,
                               kind="Internal").ap(),
        "loss_part": nc.dram_tensor("loss_part", [P, cfg.it], F32,
                                    kind="ExternalOutput").ap(),
        "nidx": nc.dram_tensor("nidx", [P, cfg.it], U32,
                               kind="ExternalOutput").ap(),
    }
    with tile.TileContext(nc) as tc:
        _body(tc, cfg, io)
    nc.compile()
    return nc


def make_in_maps(cfg: Cfg, x: np.ndarray) -> list[dict]:
    x = np.ascontiguousarray(x, dtype=np.float32)
    sim_np = ml_dtypes.float8_e4m3 if cfg.fp8 else ml_dtypes.bfloat16
    xt_q = np.ascontiguousarray(x.T.astype(sim_np))
    x_bf = x.astype(ml_dtypes.bfloat16)

    m2 = np.eye(P, dtype=np.float32)
    for c in range(P // CHUNK):
        m2[c * CHUNK:(c + 1) * CHUNK, c * CHUNK:(c + 1) * CHUNK] -= 1.0 / CHUNK
    m2b = m2.astype(ml_dtypes.bfloat16)

    pvec = np.arange(P)
    # mask tiles per it%4: -2^26 on the 4-column window, else 0
    maskdg = np.zeros((P, 4, JS), dtype=np.float32)
    for itv in range(4):
        w0 = itv * P + (pvec - pvec % GROUP)
        for off in range(GROUP):
            maskdg[pvec, itv, w0 + off] = MASKV
    maskdg_b = maskdg.reshape(P, -1).astype(ml_dtypes.bfloat16)

    # row-id base for the simr gather: p*128 + it*16
    prow = (pvec[:, None] * P + np.arange(8)[None, :] * 16).astype(np.uint32)

    in_maps = []
    for c in range(cfg.cores):
        cols = np.ones(cfg.n, dtype=bool)
        cols[c * cfg.r:(c + 1) * cfg.r] = False
        xm_nd = np.ascontiguousarray(xt_q[:, cols])          # [d, njr*JS]
        ctab2 = np.full((P, 8), 2 * c, dtype=np.uint32)

        in_maps.append({
            "xm": xm_nd,
            "xs": np.ascontiguousarray(xt_q[:, c * cfg.r:(c + 1) * cfg.r]),
            "xrb": np.ascontiguousarray(x_bf[c * cfg.r:(c + 1) * cfg.r]),
            "xfb": x_bf,
            "m2b": m2b,
            "maskdg": maskdg_b,
            "prow": prow,
            "ctab2": ctab2,
        })
    return in_maps


def reduce_outputs(cfg: Cfg, results: list[dict]) -> np.ndarray:
    total = 0.0
    for res in results:
        total += float(res["loss_part"].astype(np.float64).sum())
    return np.float32(total)


def run(cfg: Cfg, x: np.ndarray, trace: bool = False):
    nc = build(cfg)
    in_maps = make_in_maps(cfg, x)
    out = run_bass_kernel_spmd(nc, in_maps, list(range(cfg.cores)), trace=trace)
    return out


def kernel(x: np.ndarray) -> np.ndarray:
    cfg = Cfg(n=8192, d=2048, cores=8)
    last_err = None
    for _ in range(3):
        try:
            out = run(cfg, x)
            return reduce_outputs(cfg, out.results)
        except Exception as e:  # transient device errors: rebuild + retry
            last_err = e
    raise last_err
